# revision 18
# baseline (speedup 1.0000x reference)
"""Trainium2 Bass kernel for nn_ConvSkip (GNN message passing layer).

Computes, for the full graph:
    h    = data @ W_lin                 (b_lin cancels in the laplacian)
    lap  = h - (1/deg) * sum_{j in N(i)} h_j
    out  = relu(lap + merge @ W_tr + b_tr)

Sharding: nodes are sharded contiguously across 8 cores. Each core receives
the full `data` tensor (bf16, host-transposed to [128, N] and rotated so its
own shard is at rows [0, SHARD)) and redundantly computes the full
transformed feature table h (bf16) into its own DRAM. Neighbor rows are
fetched with ONE batched indirect DMA per chunk (a [128, CC] offset AP =
14336 row indices in a single instruction) and the 16-way neighbor sum is
done as accumulating selection matmuls on the tensor engine (host
pre-arranges edge order so slot->node mapping is a fixed set of 16 static
0/1 matrices). Host-side transposition of data/merge lets every matmul use
the DMA-loaded tile directly as lhsT, so the kernel issues no PE transposes.
"""

import numpy as np

P = 128
N_NODES = 50000
DEG = 16
D_IN = 128
D_OUT = 64
N_CORES = 8
SHARD = N_NODES // N_CORES  # 6250

T1 = 8                      # phase-1 supertile: node-tiles per step
N_ST1 = 49                  # supertiles covering all nodes
PAD_N = N_ST1 * T1 * P      # 50176

CH = 7                      # phase-2: node-tiles per chunk
N_CH = 7                    # chunks covering the shard (49 tiles)
PAD_S = N_CH * CH * P       # 6272
CC = CH * 16                # gather columns per chunk (112)
OWN_ST1 = (PAD_S + T1 * P - 1) // (T1 * P)  # 7 phase-1 supertiles cover shard
N_ST1_TILES = N_ST1 * T1    # 392 node-tiles; h_table row-permutation factor


def build_nc(repeat=1):
    import concourse.bass as bass
    import concourse.tile as tile
    from concourse import bacc, mybir

    f32 = mybir.dt.float32
    bf16 = mybir.dt.bfloat16
    i32 = mybir.dt.int32

    nc = bacc.Bacc("TRN2", target_bir_lowering=False)

    data_t = nc.declare_dram_parameter("data_t", [P, PAD_N], bf16, isOutput=False)
    merge_t = nc.declare_dram_parameter("merge_t", [P, PAD_S], bf16, isOutput=False)
    idx_r = nc.declare_dram_parameter("idx_r", [P, N_CH * CC], i32, isOutput=False)
    w_lin = nc.declare_dram_parameter("w_lin", [D_IN, D_OUT], bf16, isOutput=False)
    w_tr = nc.declare_dram_parameter("w_tr", [D_IN, D_OUT], bf16, isOutput=False)
    b_tr = nc.declare_dram_parameter("b_tr", [D_OUT], f32, isOutput=False)
    s_base = nc.declare_dram_parameter("s_base", [P, 16 * P], bf16, isOutput=False)
    out_r = nc.declare_dram_parameter("out_r", [PAD_S, D_OUT], f32, isOutput=True)

    with tile.TileContext(nc) as tc:
        with (
            tc.tile_pool(name="const", bufs=1) as cpool,
            tc.tile_pool(name="own", bufs=2) as opool,
            tc.tile_pool(name="sbuf", bufs=2) as pool,
            tc.tile_pool(name="ld", bufs=3) as ldpool,
            tc.tile_pool(name="psum", bufs=2, space="PSUM") as psum,
            tc.tile_pool(name="dram", bufs=2, space="DRAM") as dpool,
        ):
            # ---- constants ----
            w_lin_sb = cpool.tile([P, D_OUT], bf16)
            nc.sync.dma_start(out=w_lin_sb[:], in_=w_lin[:, :])
            w_tr_sb = cpool.tile([P, D_OUT], bf16)
            nc.sync.dma_start(out=w_tr_sb[:], in_=w_tr[:, :])
            s_base_sb = cpool.tile([P, 16, P], bf16)
            nc.sync.dma_start(
                out=s_base_sb[:], in_=s_base[:, :].rearrange("p (g m) -> p g m", g=16)
            )
            ones1 = cpool.tile([1, P], f32)
            nc.vector.memset(ones1[:], 1.0)
            btr_sb = cpool.tile([1, D_OUT], f32)
            nc.sync.dma_start(out=btr_sb[:], in_=b_tr[None, :])
            btr_t2 = cpool.tile([1, CH, D_OUT], f32)
            nc.vector.tensor_copy(
                out=btr_t2[:], in_=btr_sb[:, None, :].to_broadcast([1, CH, D_OUT])
            )
            idx_all = cpool.tile([P, N_CH * CC], i32)
            nc.sync.dma_start(out=idx_all[:], in_=idx_r[:, :])

            def phase1_st(h_table, h_own, st):
                # ---- phase 1, one supertile: h rows for T1 node-tiles ----
                r0 = st * T1 * P
                x_sb = ldpool.tile([P, T1, P], bf16, tag="ld1")
                nc.sync.dma_start(
                    out=x_sb[:],
                    in_=data_t[:, r0 : r0 + T1 * P].rearrange(
                        "k (t n) -> k t n", t=T1
                    ),
                )
                h_ps = psum.tile([P, 512], f32, tag="mm")
                for t in range(T1):
                    nc.tensor.matmul(
                        out=h_ps[:, t * D_OUT : (t + 1) * D_OUT],
                        lhsT=x_sb[:, t, :],
                        rhs=w_lin_sb[:],
                        start=True,
                        stop=True,
                    )
                if st < OWN_ST1:
                    if st % 2 == 0:
                        nc.scalar.copy(
                            out=h_own[:, st * T1 : (st + 1) * T1, :],
                            in_=h_ps[:].rearrange("p (t f) -> p t f", t=T1),
                        )
                    else:
                        nc.vector.tensor_copy(
                            out=h_own[:, st * T1 : (st + 1) * T1, :],
                            in_=h_ps[:].rearrange("p (t f) -> p t f", t=T1),
                        )
                h_sb = pool.tile([P, T1, D_OUT], bf16, tag="hcast")
                if st % 2 == 0:
                    nc.vector.tensor_copy(
                        out=h_sb[:], in_=h_ps[:].rearrange("p (t f) -> p t f", t=T1)
                    )
                else:
                    nc.scalar.copy(
                        out=h_sb[:], in_=h_ps[:].rearrange("p (t f) -> p t f", t=T1)
                    )
                # h_table rows are partition-major permuted (row sigma(n) =
                # (n%128)*N_TILES + n//128, idx pre-permuted on host) so
                # each partition writes its T1 tile-rows contiguously:
                # 128 x 1KB descriptors instead of 1024 x 128B.
                nc.sync.dma_start(
                    out=h_table[:, :].rearrange("(p T) f -> p T f", p=P)[
                        :, st * T1 : (st + 1) * T1, :
                    ],
                    in_=h_sb[:],
                )

            def phase2_chunk(h_table, h_own, ch):
                # ---- phase 2, one chunk: gather + laplacian + skip ----
                if True:
                    gath = pool.tile([P, CC, D_OUT], bf16, tag="gath", bufs=4)
                    for c in range(CC):
                        k = ch * CC + c
                        nc.gpsimd.indirect_dma_start(
                            out=gath[:, c, :],
                            out_offset=None,
                            in_=h_table[:, :],
                            in_offset=bass.IndirectOffsetOnAxis(
                                ap=idx_all[:, k : k + 1], axis=0
                            ),
                        )
                    # neighbor-sum via accumulating selection matmuls
                    nsum_ps = psum.tile([P, CH, D_OUT], f32, tag="ns")
                    for g in range(16):
                        nc.tensor.matmul(
                            out=nsum_ps[:, :, :].rearrange("m c f -> m (c f)"),
                            lhsT=s_base_sb[:, g, :],
                            rhs=gath[:, g * CH : (g + 1) * CH, :].rearrange(
                                "p c f -> p (c f)"
                            ),
                            start=(g == 0),
                            stop=(g == 15),
                        )
                    # skip branch
                    m_sb = ldpool.tile([P, CH, P], bf16, tag="ldm")
                    r0 = ch * CH * P
                    nc.sync.dma_start(
                        out=m_sb[:],
                        in_=merge_t[:, r0 : r0 + CH * P].rearrange(
                            "k (t n) -> k t n", t=CH
                        ),
                    )
                    sk_ps = psum.tile([P, 512], f32, tag="mm")
                    nc.tensor.matmul(
                        out=sk_ps[:, : CH * D_OUT],
                        lhsT=ones1[:],
                        rhs=btr_t2[:].rearrange("o t f -> o (t f)"),
                        start=True,
                        stop=False,
                        skip_group_check=True,
                    )
                    for t in range(CH):
                        nc.tensor.matmul(
                            out=sk_ps[:, t * D_OUT : (t + 1) * D_OUT],
                            lhsT=m_sb[:, t, :],
                            rhs=w_tr_sb[:],
                            start=False,
                            stop=(t == CH - 1),
                            skip_group_check=True,
                        )
                    # combine: relu(h_own - nsum/deg + skip)
                    tmp = pool.tile([P, CH, D_OUT], f32, tag="tmp")
                    nc.vector.scalar_tensor_tensor(
                        out=tmp[:],
                        in0=nsum_ps[:],
                        scalar=-1.0 / DEG,
                        in1=h_own[:, ch * CH : (ch + 1) * CH, :],
                        op0=mybir.AluOpType.mult,
                        op1=mybir.AluOpType.add,
                    )
                    osum = pool.tile([P, CH, D_OUT], f32, tag="osum")
                    nc.vector.tensor_tensor(
                        out=osum[:],
                        in0=tmp[:],
                        in1=sk_ps[:, : CH * D_OUT].rearrange("p (t f) -> p t f", t=CH),
                        op=mybir.AluOpType.add,
                    )
                    orelu = pool.tile([P, CH, D_OUT], f32, tag="orelu")
                    nc.scalar.activation(
                        out=orelu[:],
                        in_=osum[:],
                        func=mybir.ActivationFunctionType.Relu,
                    )
                    nc.sync.dma_start(
                        out=out_r[r0 : r0 + CH * P, :].rearrange(
                            "(t p) f -> p t f", p=P
                        ),
                        in_=orelu[:],
                    )

            # Software-pipelined repeat loop: body r+1's phase-1 supertiles
            # are interleaved between body r's phase-2 chunks so the in-order
            # PE fills its gather-wait stalls with next-body phase-1 matmuls
            # and h_table(r+1) is complete when body r's gathers drain.
            def alloc(rep):
                h_table = dpool.tile(
                    [PAD_N, D_OUT], bf16, tag="ht", name=f"ht{rep}"
                )
                h_own = opool.tile(
                    [P, OWN_ST1 * T1, D_OUT], f32, tag="hown", name=f"hown{rep}"
                )
                return h_table, h_own

            cur = alloc(0)
            for st in range(N_ST1):
                phase1_st(cur[0], cur[1], st)
            for rep in range(repeat):
                nxt = alloc(rep + 1) if rep + 1 < repeat else None
                for ch in range(N_CH):
                    phase2_chunk(cur[0], cur[1], ch)
                    if nxt is not None:
                        for k in range(N_ST1 // N_CH):
                            phase1_st(nxt[0], nxt[1], ch * (N_ST1 // N_CH) + k)
                if nxt is not None:
                    cur = nxt

    _deepen_swdge_pipeline(nc, depth=14)
    nc.finalize()
    return nc


def _deepen_swdge_pipeline(nc, depth=8):
    """Relax the tile framework's depth-1-per-lane pacing on the SWDGE
    (qPoolDynamic) gather stream.

    The scheduler round-robins gather completions over 8 DMASW semaphore
    lanes and makes each gather wait for the previous gather on its lane to
    fully complete (wait DMASW(k%8) >= 16*(k//8)), capping the pipeline at 8
    in-flight indirect DMAs and putting a full DMA round trip on the
    critical path every 8 instructions. The SWDGE descriptor carveout
    actually holds 16384 descriptors (= 128 of these 128-offset gathers), so
    letting each lane run `depth` instructions ahead (8*depth in flight,
    128*8*depth descriptors) is safe and removes the round-trip latency from
    the steady-state rate. Data-dependency waits (consumers of the gathered
    tiles, WAR on tile reuse) are untouched.
    """
    import concourse.mybir as mybir

    delta = 16 * (depth - 1)
    for blk in nc.main_func.blocks:
        for ins in blk.instructions:
            if (
                isinstance(ins, mybir.InstDMACopy)
                and getattr(ins, "queue", "") == "qPoolDynamic"
            ):
                si = ins.sync_info
                if si is None:
                    continue
                keep = []
                for w in si.on_wait:
                    name = getattr(w, "ant_name", "") or ""
                    if (
                        w.sync_type == "semaphore"
                        and name.startswith("DMASW")
                        and w.wait_mode == "sem-ge-imm"
                    ):
                        v = w.wait_value - delta
                        if v > 0:
                            w.wait_value = v
                            keep.append(w)
                    else:
                        keep.append(w)
                si.on_wait = keep


def _make_in_maps(data, merge, structure, W_lin, W_tr, b_tr):
    import ml_dtypes

    bf16 = ml_dtypes.bfloat16

    data = np.ascontiguousarray(data, dtype=np.float32)
    merge = np.ascontiguousarray(merge, dtype=np.float32)
    structure = np.asarray(structure, dtype=np.int64)
    W_lin_b = np.ascontiguousarray(W_lin, dtype=np.float32).astype(bf16)
    W_tr_b = np.ascontiguousarray(W_tr, dtype=np.float32).astype(bf16)
    b_tr = np.ascontiguousarray(b_tr, dtype=np.float32)

    # S_all[p, g, m] = 1 iff m == g*8 + p%8 (selection matrices, one per group)
    s_base = np.zeros((P, 16, P), dtype=bf16)
    for p in range(P):
        for g in range(16):
            s_base[p, g, g * 8 + p % 8] = 1.0
    s_base = s_base.reshape(P, 16 * P)

    in_maps = []
    for k in range(N_CORES):
        lo = k * SHARD
        d = np.zeros((PAD_N, D_IN), dtype=bf16)
        d[:N_NODES] = np.roll(data, -lo, axis=0).astype(bf16)
        d_t = np.ascontiguousarray(d.T)
        m = np.zeros((PAD_S, D_IN), dtype=bf16)
        m[:SHARD] = merge[lo : lo + SHARD].astype(bf16)
        m_t = np.ascontiguousarray(m.T)

        idxr = np.zeros((PAD_S, DEG), dtype=np.int64)
        idxr[:SHARD] = (structure[lo : lo + SHARD] - lo) % N_NODES
        # edge reorder: column c = g*CH + t_rel (per chunk), partition
        # p = s*8 + nlo; node = (ch*CH + t_rel)*128 + g*8 + nlo, slot = s
        E = idxr.reshape(N_CH * CH, 16, 8, DEG)       # [t, g, nlo, s]
        E = E.transpose(0, 1, 3, 2)                   # [t, g, s, nlo]
        E = E.reshape(N_CH, CH, 16, P)                # [ch, t_rel, g, p]
        E = E.transpose(0, 2, 1, 3)                   # [ch, g, t_rel, p]
        # idx32[p, ch*CC + c] with c = g*CH + t_rel
        idx32 = (
            E.reshape(N_CH * CC, P).T.astype(np.int32)
        )
        # apply the partition-major h_table row permutation sigma(n)
        idx32 = (idx32 % P) * N_ST1_TILES + idx32 // P
        in_maps.append(
            {
                "data_t": d_t,
                "merge_t": m_t,
                "idx_r": np.ascontiguousarray(idx32),
                "w_lin": W_lin_b,
                "w_tr": W_tr_b,
                "b_tr": b_tr,
                "s_base": s_base,
            }
        )
    return in_maps


_NC_CACHE = {}


def _get_nc():
    if "nc" not in _NC_CACHE:
        _NC_CACHE["nc"] = build_nc()
    return _NC_CACHE["nc"]


def kernel(data, merge, structure, W_lin, b_lin, W_tr, b_tr):
    from concourse.bass_utils import run_bass_kernel_spmd

    del b_lin  # cancels exactly in the normalized laplacian
    nc = _get_nc()
    in_maps = _make_in_maps(
        np.asarray(data),
        np.asarray(merge),
        np.asarray(structure),
        np.asarray(W_lin),
        np.asarray(W_tr),
        np.asarray(b_tr),
    )
    res = run_bass_kernel_spmd(nc, in_maps, core_ids=list(range(N_CORES)))
    global LAST_RESULTS
    LAST_RESULTS = res
    out = np.concatenate(
        [np.asarray(res.results[k]["out_r"])[:SHARD] for k in range(N_CORES)], axis=0
    )
    return out.astype(np.float32)


# revision 20
# speedup vs baseline: 1.0477x; 1.0477x over previous
"""Trainium2 Bass kernel for nn_ConvSkip (GNN message passing layer).

Computes, for the full graph:
    h    = data @ W_lin                 (b_lin cancels in the laplacian)
    lap  = h - (1/deg) * sum_{j in N(i)} h_j
    out  = relu(lap + merge @ W_tr + b_tr)

Sharding: nodes are sharded contiguously across 8 cores. Each core receives
the full `data` tensor (bf16, host-transposed to [128, N] and rotated so its
own shard is at rows [0, SHARD)) and redundantly computes the full
transformed feature table h (bf16) into its own DRAM. Neighbor rows are
fetched with ONE batched indirect DMA per chunk (a [128, CC] offset AP =
14336 row indices in a single instruction) and the 16-way neighbor sum is
done as accumulating selection matmuls on the tensor engine (host
pre-arranges edge order so slot->node mapping is a fixed set of 16 static
0/1 matrices). Host-side transposition of data/merge lets every matmul use
the DMA-loaded tile directly as lhsT, so the kernel issues no PE transposes.
"""

import numpy as np

P = 128
N_NODES = 50000
DEG = 16
D_IN = 128
D_OUT = 64
N_CORES = 8
SHARD = N_NODES // N_CORES  # 6250

T1 = 8                      # phase-1 supertile: node-tiles per step
N_ST1 = 49                  # supertiles covering all nodes
PAD_N = N_ST1 * T1 * P      # 50176

CH = 7                      # phase-2: node-tiles per chunk
N_CH = 7                    # chunks covering the shard (49 tiles)
PAD_S = N_CH * CH * P       # 6272
CC = CH * 16                # gather columns per chunk (112)
OWN_ST1 = (PAD_S + T1 * P - 1) // (T1 * P)  # 7 phase-1 supertiles cover shard
N_ST1_TILES = N_ST1 * T1    # 392 node-tiles; h_table row-permutation factor


def build_nc(repeat=1):
    import concourse.bass as bass
    import concourse.tile as tile
    from concourse import bacc, mybir

    f32 = mybir.dt.float32
    bf16 = mybir.dt.bfloat16
    i32 = mybir.dt.int32

    nc = bacc.Bacc("TRN2", target_bir_lowering=False)

    data_t = nc.declare_dram_parameter("data_t", [P, PAD_N], bf16, isOutput=False)
    merge_t = nc.declare_dram_parameter("merge_t", [P, PAD_S], bf16, isOutput=False)
    idx_r = nc.declare_dram_parameter("idx_r", [P, N_CH * CC], i32, isOutput=False)
    w_lin = nc.declare_dram_parameter("w_lin", [D_IN, D_OUT], bf16, isOutput=False)
    w_tr = nc.declare_dram_parameter("w_tr", [D_IN, D_OUT], bf16, isOutput=False)
    b_tr = nc.declare_dram_parameter("b_tr", [D_OUT], f32, isOutput=False)
    s_base = nc.declare_dram_parameter("s_base", [P, 16 * P], bf16, isOutput=False)
    out_r = nc.declare_dram_parameter("out_r", [PAD_S, D_OUT], f32, isOutput=True)

    with tile.TileContext(nc) as tc:
        with (
            tc.tile_pool(name="const", bufs=1) as cpool,
            tc.tile_pool(name="own", bufs=2) as opool,
            tc.tile_pool(name="sbuf", bufs=2) as pool,
            tc.tile_pool(name="ld", bufs=3) as ldpool,
            tc.tile_pool(name="psum", bufs=2, space="PSUM") as psum,
            tc.tile_pool(name="dram", bufs=2, space="DRAM") as dpool,
        ):
            # ---- constants ----
            w_lin_sb = cpool.tile([P, D_OUT], bf16)
            nc.sync.dma_start(out=w_lin_sb[:], in_=w_lin[:, :])
            w_tr_sb = cpool.tile([P, D_OUT], bf16)
            nc.sync.dma_start(out=w_tr_sb[:], in_=w_tr[:, :])
            s_base_sb = cpool.tile([P, 16, P], bf16)
            nc.sync.dma_start(
                out=s_base_sb[:], in_=s_base[:, :].rearrange("p (g m) -> p g m", g=16)
            )
            ones1 = cpool.tile([1, P], f32)
            nc.vector.memset(ones1[:], 1.0)
            btr_sb = cpool.tile([1, D_OUT], f32)
            nc.sync.dma_start(out=btr_sb[:], in_=b_tr[None, :])
            btr_t2 = cpool.tile([1, CH, D_OUT], f32)
            nc.vector.tensor_copy(
                out=btr_t2[:], in_=btr_sb[:, None, :].to_broadcast([1, CH, D_OUT])
            )
            idx_all = cpool.tile([P, N_CH * CC], i32)
            nc.sync.dma_start(out=idx_all[:], in_=idx_r[:, :])

            def phase1_st(h_table, h_own, st):
                # ---- phase 1, one supertile: h rows for T1 node-tiles ----
                r0 = st * T1 * P
                x_sb = ldpool.tile([P, T1, P], bf16, tag="ld1")
                nc.sync.dma_start(
                    out=x_sb[:],
                    in_=data_t[:, r0 : r0 + T1 * P].rearrange(
                        "k (t n) -> k t n", t=T1
                    ),
                )
                h_ps = psum.tile([P, 512], f32, tag="mm")
                for t in range(T1):
                    nc.tensor.matmul(
                        out=h_ps[:, t * D_OUT : (t + 1) * D_OUT],
                        lhsT=x_sb[:, t, :],
                        rhs=w_lin_sb[:],
                        start=True,
                        stop=True,
                    )
                if st < OWN_ST1:
                    if st % 2 == 0:
                        nc.scalar.copy(
                            out=h_own[:, st * T1 : (st + 1) * T1, :],
                            in_=h_ps[:].rearrange("p (t f) -> p t f", t=T1),
                        )
                    else:
                        nc.vector.tensor_copy(
                            out=h_own[:, st * T1 : (st + 1) * T1, :],
                            in_=h_ps[:].rearrange("p (t f) -> p t f", t=T1),
                        )
                h_sb = pool.tile([P, T1, D_OUT], bf16, tag="hcast")
                if st % 2 == 0:
                    nc.vector.tensor_copy(
                        out=h_sb[:], in_=h_ps[:].rearrange("p (t f) -> p t f", t=T1)
                    )
                else:
                    nc.scalar.copy(
                        out=h_sb[:], in_=h_ps[:].rearrange("p (t f) -> p t f", t=T1)
                    )
                # h_table rows are partition-major permuted (row sigma(n) =
                # (n%128)*N_TILES + n//128, idx pre-permuted on host) so
                # each partition writes its T1 tile-rows contiguously:
                # 128 x 1KB descriptors instead of 1024 x 128B.
                nc.sync.dma_start(
                    out=h_table[:, :].rearrange("(p T) f -> p T f", p=P)[
                        :, st * T1 : (st + 1) * T1, :
                    ],
                    in_=h_sb[:],
                )

            def phase2_chunk(h_table, h_own, ch):
                # ---- phase 2, one chunk: gather + laplacian + skip ----
                if True:
                    gath = pool.tile([P, CC, D_OUT], bf16, tag="gath", bufs=3)
                    for c in range(CC):
                        k = ch * CC + c
                        nc.gpsimd.indirect_dma_start(
                            out=gath[:, c, :],
                            out_offset=None,
                            in_=h_table[:, :],
                            in_offset=bass.IndirectOffsetOnAxis(
                                ap=idx_all[:, k : k + 1], axis=0
                            ),
                        )
                    # neighbor-sum via accumulating selection matmuls
                    nsum_ps = psum.tile([P, CH, D_OUT], f32, tag="ns")
                    for g in range(16):
                        nc.tensor.matmul(
                            out=nsum_ps[:, :, :].rearrange("m c f -> m (c f)"),
                            lhsT=s_base_sb[:, g, :],
                            rhs=gath[:, g * CH : (g + 1) * CH, :].rearrange(
                                "p c f -> p (c f)"
                            ),
                            start=(g == 0),
                            stop=(g == 15),
                        )
                    # skip branch
                    m_sb = ldpool.tile([P, CH, P], bf16, tag="ldm")
                    r0 = ch * CH * P
                    nc.sync.dma_start(
                        out=m_sb[:],
                        in_=merge_t[:, r0 : r0 + CH * P].rearrange(
                            "k (t n) -> k t n", t=CH
                        ),
                    )
                    sk_ps = psum.tile([P, 512], f32, tag="mm")
                    nc.tensor.matmul(
                        out=sk_ps[:, : CH * D_OUT],
                        lhsT=ones1[:],
                        rhs=btr_t2[:].rearrange("o t f -> o (t f)"),
                        start=True,
                        stop=False,
                        skip_group_check=True,
                    )
                    for t in range(CH):
                        nc.tensor.matmul(
                            out=sk_ps[:, t * D_OUT : (t + 1) * D_OUT],
                            lhsT=m_sb[:, t, :],
                            rhs=w_tr_sb[:],
                            start=False,
                            stop=(t == CH - 1),
                            skip_group_check=True,
                        )
                    # combine: relu(h_own - nsum/deg + skip)
                    tmp = pool.tile([P, CH, D_OUT], f32, tag="tmp")
                    nc.vector.scalar_tensor_tensor(
                        out=tmp[:],
                        in0=nsum_ps[:],
                        scalar=-1.0 / DEG,
                        in1=h_own[:, ch * CH : (ch + 1) * CH, :],
                        op0=mybir.AluOpType.mult,
                        op1=mybir.AluOpType.add,
                    )
                    osum = pool.tile([P, CH, D_OUT], f32, tag="osum")
                    nc.vector.tensor_tensor(
                        out=osum[:],
                        in0=tmp[:],
                        in1=sk_ps[:, : CH * D_OUT].rearrange("p (t f) -> p t f", t=CH),
                        op=mybir.AluOpType.add,
                    )
                    orelu = pool.tile([P, CH, D_OUT], f32, tag="orelu")
                    nc.scalar.activation(
                        out=orelu[:],
                        in_=osum[:],
                        func=mybir.ActivationFunctionType.Relu,
                    )
                    nc.sync.dma_start(
                        out=out_r[r0 : r0 + CH * P, :].rearrange(
                            "(t p) f -> p t f", p=P
                        ),
                        in_=orelu[:],
                    )

            # Software-pipelined repeat loop: body r+1's phase-1 supertiles
            # are interleaved between body r's phase-2 chunks so the in-order
            # PE fills its gather-wait stalls with next-body phase-1 matmuls
            # and h_table(r+1) is complete when body r's gathers drain.
            def alloc(rep):
                h_table = dpool.tile(
                    [PAD_N, D_OUT], bf16, tag="ht", name=f"ht{rep}"
                )
                h_own = opool.tile(
                    [P, OWN_ST1 * T1, D_OUT], f32, tag="hown", name=f"hown{rep}"
                )
                return h_table, h_own

            cur = alloc(0)
            for st in range(N_ST1):
                phase1_st(cur[0], cur[1], st)
            for rep in range(repeat):
                nxt = alloc(rep + 1) if rep + 1 < repeat else None
                for ch in range(N_CH):
                    phase2_chunk(cur[0], cur[1], ch)
                    if nxt is not None:
                        for k in range(N_ST1 // N_CH):
                            phase1_st(nxt[0], nxt[1], ch * (N_ST1 // N_CH) + k)
                if nxt is not None:
                    cur = nxt

    _deepen_swdge_pipeline(nc, depth=12)
    nc.finalize()
    return nc


def _deepen_swdge_pipeline(nc, depth=8):
    """Relax the tile framework's depth-1-per-lane pacing on the SWDGE
    (qPoolDynamic) gather stream.

    The scheduler round-robins gather completions over 8 DMASW semaphore
    lanes and makes each gather wait for the previous gather on its lane to
    fully complete (wait DMASW(k%8) >= 16*(k//8)), capping the pipeline at 8
    in-flight indirect DMAs and putting a full DMA round trip on the
    critical path every 8 instructions. The SWDGE descriptor carveout
    actually holds 16384 descriptors (= 128 of these 128-offset gathers), so
    letting each lane run `depth` instructions ahead (8*depth in flight,
    128*8*depth descriptors) is safe and removes the round-trip latency from
    the steady-state rate. Data-dependency waits (consumers of the gathered
    tiles, WAR on tile reuse) are untouched.
    """
    import concourse.mybir as mybir

    delta = 16 * (depth - 1)
    for blk in nc.main_func.blocks:
        for ins in blk.instructions:
            if (
                isinstance(ins, mybir.InstDMACopy)
                and getattr(ins, "queue", "") == "qPoolDynamic"
            ):
                si = ins.sync_info
                if si is None:
                    continue
                keep = []
                for w in si.on_wait:
                    name = getattr(w, "ant_name", "") or ""
                    if (
                        w.sync_type == "semaphore"
                        and name.startswith("DMASW")
                        and w.wait_mode == "sem-ge-imm"
                    ):
                        v = w.wait_value - delta
                        if v > 0:
                            w.wait_value = v
                            keep.append(w)
                    else:
                        keep.append(w)
                si.on_wait = keep


def _make_in_maps(data, merge, structure, W_lin, W_tr, b_tr):
    import ml_dtypes

    bf16 = ml_dtypes.bfloat16

    data = np.ascontiguousarray(data, dtype=np.float32)
    merge = np.ascontiguousarray(merge, dtype=np.float32)
    structure = np.asarray(structure, dtype=np.int64)
    W_lin_b = np.ascontiguousarray(W_lin, dtype=np.float32).astype(bf16)
    W_tr_b = np.ascontiguousarray(W_tr, dtype=np.float32).astype(bf16)
    b_tr = np.ascontiguousarray(b_tr, dtype=np.float32)

    # S_all[p, g, m] = 1 iff m == g*8 + p%8 (selection matrices, one per group)
    s_base = np.zeros((P, 16, P), dtype=bf16)
    for p in range(P):
        for g in range(16):
            s_base[p, g, g * 8 + p % 8] = 1.0
    s_base = s_base.reshape(P, 16 * P)

    in_maps = []
    for k in range(N_CORES):
        lo = k * SHARD
        d = np.zeros((PAD_N, D_IN), dtype=bf16)
        d[:N_NODES] = np.roll(data, -lo, axis=0).astype(bf16)
        d_t = np.ascontiguousarray(d.T)
        m = np.zeros((PAD_S, D_IN), dtype=bf16)
        m[:SHARD] = merge[lo : lo + SHARD].astype(bf16)
        m_t = np.ascontiguousarray(m.T)

        idxr = np.zeros((PAD_S, DEG), dtype=np.int64)
        idxr[:SHARD] = (structure[lo : lo + SHARD] - lo) % N_NODES
        # edge reorder: column c = g*CH + t_rel (per chunk), partition
        # p = s*8 + nlo; node = (ch*CH + t_rel)*128 + g*8 + nlo, slot = s
        E = idxr.reshape(N_CH * CH, 16, 8, DEG)       # [t, g, nlo, s]
        E = E.transpose(0, 1, 3, 2)                   # [t, g, s, nlo]
        E = E.reshape(N_CH, CH, 16, P)                # [ch, t_rel, g, p]
        E = E.transpose(0, 2, 1, 3)                   # [ch, g, t_rel, p]
        # idx32[p, ch*CC + c] with c = g*CH + t_rel
        idx32 = (
            E.reshape(N_CH * CC, P).T.astype(np.int32)
        )
        # apply the partition-major h_table row permutation sigma(n)
        idx32 = (idx32 % P) * N_ST1_TILES + idx32 // P
        in_maps.append(
            {
                "data_t": d_t,
                "merge_t": m_t,
                "idx_r": np.ascontiguousarray(idx32),
                "w_lin": W_lin_b,
                "w_tr": W_tr_b,
                "b_tr": b_tr,
                "s_base": s_base,
            }
        )
    return in_maps


_NC_CACHE = {}


def _get_nc():
    if "nc" not in _NC_CACHE:
        _NC_CACHE["nc"] = build_nc()
    return _NC_CACHE["nc"]


def kernel(data, merge, structure, W_lin, b_lin, W_tr, b_tr):
    from concourse.bass_utils import run_bass_kernel_spmd

    del b_lin  # cancels exactly in the normalized laplacian
    nc = _get_nc()
    in_maps = _make_in_maps(
        np.asarray(data),
        np.asarray(merge),
        np.asarray(structure),
        np.asarray(W_lin),
        np.asarray(W_tr),
        np.asarray(b_tr),
    )
    res = run_bass_kernel_spmd(nc, in_maps, core_ids=list(range(N_CORES)))
    global LAST_RESULTS
    LAST_RESULTS = res
    out = np.concatenate(
        [np.asarray(res.results[k]["out_r"])[:SHARD] for k in range(N_CORES)], axis=0
    )
    return out.astype(np.float32)
